# revision 39
# baseline (speedup 1.0000x reference)
"""AssignAttention (hard-routing slot attention) Trainium2 kernel, 8-core data-parallel.

Problem: B=16, N=64 groups, S=4096 tokens, C=768, H=8 heads, HD=96.
  q = query @ Wq.T; k = key @ Wk.T (per-head split)
  logits = q @ k.T; hard-argmax over the 64 groups per token -> one-hot
  (softmax and the *SCALE factor are argmax-invariant, so both are skipped);
  attn = onehot / (count + 1); out = (attn @ v per head) @ Wo.T + bo

Sharding: data-parallel over batch B: 16 batches / 8 cores = 2 per core.
No collectives; the host concatenates per-core outputs.

Algorithm per core (validated vs the fp32 reference):
  - Logits REASSOCIATED: Y[c, (h,n)] = sum_d Wk[d(h), c] qp[n, d] is tiny
    (per batch 768x512), and logits[s, (h,n)] = sum_c keyT[c, s] Y[c, (h,n)]
    -- the dominant k-projection matmul disappears entirely.
  - Logits precision: fp16 main term (keeps FWL weight loads, which fp32r
    weights would disable) plus TWO fp8e5m2 DoubleRow correction matmuls
    capturing the fp16 rounding residuals of each operand:
      K@Y ~= Kh@Yh + (16*Kl)@(Yh/16) + (Kh/16)@(16*Yl)
    where Kh = fp16(K), Kl = fp16(K - Kh) etc. The +-2^4 pre-scaling keeps
    the e5m2 operands in normal range (subnormal flush would cost argmax
    flips). 6 fp16 MMs + 6 DR MMs (K=256 packed pairs of c-tiles) per
    128-token subtile vs 18 fp16 MMs for the hi/lo x3 split,
    logit rel err ~4e-5 (~27 argmax flips over the whole problem, ~5e-3
    final rel_l2 vs the 2e-2 gate; the exact fp32 q-projection buys extra
    margin over the fp16x3-q sim).
  - Per-subtile operand prep (PE fp32 transpose -> ACT fp16 copy ->
    DVE residual subtract -> ACT scaled fp8 casts) is software-pipelined
    two subtiles ahead of the logits matmuls (three warm-started before
    the weight phase) so the PE never waits on the ACT/DVE chain.
  - argmax via row-max + (x >= max) compare; counts via a ones-column in
    the group-sum rhs; renorm = per-partition reciprocal. Group-sum pass 1
    is emitted one subtile behind the logits so the argmax chain hides
    under the next subtile's matmuls instead of stalling the in-order PE
    queue (PSUM accumulation is add-commutative).
  - v-path reassociated: group-sums taken on raw fp16 keys
    (gs_raw[n,c] = onehot^T @ key16), divided by (count+1), and only the
    64 group vectors are projected through WvT. The fp16 natural-layout
    key (with a leading ones column) is RETAINED in SBUF for the whole
    batch, so group-sum pass 2 (c half two) needs no HBM re-stream and no
    re-cast.
  - Both batches are PACKED into M=128 wherever the row dim would be 64:
    q path (q/qT/q-proj), Y (head-quad PSUM groups over both batches), the
    v-projection and Wo projection, halving the small-matmul count. The
    bias is a partition-broadcast DMA tile folded into the PSUM->SBUF move
    on the DVE (no K=1 matmuls); output stores are split per batch x
    column-half across both HWDGE rings and issued as each half lands.
  - Weight DMAs split across the sync/scalar rings; first three key
    chunks prefetched on the gpsimd (SWDGE) queue during weight prep.
  - One accumulation group per 2KB PSUM bank; all matmul row offsets 0.
"""

import sys

if "/opt/trn_rl_repo" not in sys.path:
    sys.path.insert(0, "/opt/trn_rl_repo")

import numpy as np

import concourse.bass as bass
import concourse.mybir as mybir
from concourse import bacc
import concourse.tile as tile
from concourse.masks import make_identity

f32 = mybir.dt.float32
f32r = mybir.dt.float32r
f16 = mybir.dt.float16
f8 = mybir.dt.float8e5
DRMODE = mybir.MatmulPerfMode.DoubleRow

C = 768
H = 8
HD = 96
NG = 64  # groups
CT = C // 128  # 6 c-tiles
CP = 128 * H  # d-padded width (head h at 128h..128h+96)
S_CHUNK = 256


def build_nc(b_sh=2, S=4096):
    nc = bacc.Bacc()

    query_d = nc.declare_dram_parameter("query", [b_sh, NG, C], f32, isOutput=False)
    key_d = nc.declare_dram_parameter("key_in", [b_sh, S, C], f32, isOutput=False)
    wq_d = nc.declare_dram_parameter("Wq", [C, C], f32, isOutput=False)
    wk_d = nc.declare_dram_parameter("Wk", [C, C], f32, isOutput=False)
    wv_d = nc.declare_dram_parameter("Wv", [C, C], f32, isOutput=False)
    wo_d = nc.declare_dram_parameter("Wo", [C, C], f32, isOutput=False)
    bo_d = nc.declare_dram_parameter("bo", [C], f32, isOutput=False)
    out_d = nc.declare_dram_parameter("out", [b_sh, NG, C], f32, isOutput=True)

    n_chunks = S // S_CHUNK
    n_sub = S_CHUNK // 128
    NSUB = S // 128
    MB = b_sh * NG  # 128 packed rows (2 batches x 64 groups)
    ENGS = None  # set inside

    from contextlib import ExitStack

    with tile.TileContext(nc) as tc:
        with (
            tc.tile_pool(name="wconst", bufs=1) as wconst,
            tc.tile_pool(name="kin", bufs=4) as kin,
            tc.tile_pool(name="mxp", bufs=3) as mxp,
            tc.tile_pool(name="ps_a", bufs=2, space="PSUM") as ps_a,
            tc.tile_pool(name="ps_tr", bufs=2, space="PSUM") as ps_tr,
            tc.tile_pool(name="ps_g4", bufs=4, space="PSUM") as ps_g4,
        ):
            ENGS = (nc.sync, nc.scalar, nc.gpsimd)
            # ---- constants ----
            ident128_32 = wconst.tile([128, 128], f32)
            make_identity(nc, ident128_32[:])
            ident128_16 = wconst.tile([128, 128], f16)
            make_identity(nc, ident128_16[:])
            bo_bc = wconst.tile([128, C], f32)
            nc.gpsimd.dma_start(
                out=bo_bc[:], in_=bo_d[:].unsqueeze(0).to_broadcast((128, C))
            )

            # persistent weight / Y tiles
            wvT_h = wconst.tile([128, CT, C], f16)
            woT_h = wconst.tile([128, CT, C], f16)
            Yh = wconst.tile([128, b_sh, CT, 512], f16)
            y8a = wconst.tile([128, b_sh, CT, 512], f8)
            y8b = wconst.tile([128, b_sh, CT, 512], f8)

            # prefetch first key chunks of batch 0 on the SWDGE queue
            knat_tiles = {}
            def fetch_chunk(b, c):
                if (b, c) in knat_tiles or c >= n_chunks:
                    return
                t = kin.tile([128, n_sub, C], f32, tag="knat", name=f"knat_{b}_{c}")
                nc.gpsimd.dma_start(
                    out=t[:],
                    in_=key_d[b, c * S_CHUNK : (c + 1) * S_CHUNK, :].rearrange(
                        "(i p) c -> p i c", p=128
                    ),
                )
                knat_tiles[(b, c)] = t

            for c0 in range(min(4, n_chunks)):
                fetch_chunk(0, c0)

            # per-subtile logits-operand prep; kTp is open before phase 1 so
            # the first subtiles' transposes/casts can run during phase-1
            # weight-DMA stalls (k16 cast deferred for those warm subtiles)
            kTp = tc.alloc_tile_pool(name="kTp", bufs=3)

            def prep_subtile_core(b, knat, i, ig, k16):
                kTh = kTp.tile([128, CT, 128], f16, tag="kTh")
                klr16 = kTp.tile([128, CT, 128], f16, tag="klr16")
                for g in range(2):
                    trp = ps_tr.tile([128, 3, 128], f32, tag="pstr")
                    for j in range(3):
                        u = 3 * g + j
                        nc.tensor.matmul(
                            trp[:, j, :],
                            knat[:, i, 128 * u : 128 * u + 128],
                            ident128_32[:],
                            is_transpose=True,
                            start=(j == 0),
                            stop=(j == 2),
                        )
                    nc.scalar.copy(out=kTh[:, 3 * g : 3 * g + 3, :], in_=trp[:])
                    nc.vector.tensor_tensor(
                        out=klr16[:, 3 * g : 3 * g + 3, :],
                        in0=trp[:],
                        in1=kTh[:, 3 * g : 3 * g + 3, :],
                        op=mybir.AluOpType.subtract,
                    )
                klr8 = kTp.tile([128, CT, 128], f8, tag="klr8")
                nc.scalar.mul(out=klr8[:], in_=klr16[:], mul=16.0)
                kr8 = kTp.tile([128, CT, 128], f8, tag="kr8")
                nc.scalar.mul(out=kr8[:], in_=kTh[:], mul=1.0 / 16.0)
                if k16 is not None:
                    nc.vector.tensor_copy(k16[:, ig, 1:769], knat[:, i, :])
                return kTh, klr8, kr8

            # warm preps: batch-0 subtiles 0..2 (k16 cast deferred into the
            # loop so the casts don't queue on DVE ahead of the first argmax)
            warm_preps = {}
            for wig in range(min(3, NSUB)):
                chunk, i = divmod(wig, n_sub)
                warm_preps[wig] = prep_subtile_core(0, knat_tiles[(0, chunk)], i, wig, None)

            # =============== phase 1: weights + q path + Y ===============
            with (
                tc.tile_pool(name="qtmp", bufs=1) as qtmp,
                tc.tile_pool(name="wtmp", bufs=5) as wtmp,
                tc.tile_pool(name="ylrp", bufs=2) as ylrp,
            ):
                # Wq transposed to c-major fp32, d-padded
                wqT = qtmp.tile([128, CT, CP], f32)
                for hd in range(H):
                    wnat = wtmp.tile([128, C], f32, tag="wnat")
                    nc.vector.memset(wnat[HD:128, :], 0.0)
                    nc.sync.dma_start(
                        out=wnat[0 : HD // 2, :], in_=wq_d[HD * hd : HD * hd + HD // 2, :]
                    )
                    nc.scalar.dma_start(
                        out=wnat[HD // 2 : HD, :],
                        in_=wq_d[HD * hd + HD // 2 : HD * hd + HD, :],
                    )
                    for g in range(2):
                        trp = ps_tr.tile([128, 3, 128], f32, tag="pstr")
                        for j in range(3):
                            u = 3 * g + j
                            nc.tensor.matmul(
                                trp[:, j, :],
                                wnat[:, 128 * u : 128 * u + 128],
                                ident128_32[:],
                                is_transpose=True,
                                start=(j == 0),
                                stop=(j == 2),
                            )
                        nc.scalar.copy(
                            out=wqT[:, 3 * g : 3 * g + 3, 128 * hd : 128 * hd + 128],
                            in_=trp[:],
                        )

                # query (both batches packed), transposed to c-major fp32
                q_nat = qtmp.tile([MB, C], f32)
                qflat = query_d[:].rearrange("b n c -> (b n) c")
                nc.sync.dma_start(out=q_nat[0:64, :], in_=qflat[0:64, :])
                nc.scalar.dma_start(out=q_nat[64:128, :], in_=qflat[64:128, :])
                qTq = qtmp.tile([128, CT, MB], f32)
                for g in range(2):
                    trp = ps_tr.tile([128, 3, 128], f32, tag="pstr")
                    for j in range(3):
                        u = 3 * g + j
                        nc.tensor.matmul(
                            trp[:, j, :],
                            q_nat[:, 128 * u : 128 * u + 128],
                            ident128_32[:],
                            is_transpose=True,
                            start=(j == 0),
                            stop=(j == 2),
                        )
                    nc.scalar.copy(out=qTq[:, 3 * g : 3 * g + 3, :], in_=trp[:])

                # q projection fp32 direct (exact), M=128 packed
                q_sb = qtmp.tile([MB, CP], f32)
                for half in range(2):
                    nsl = slice(512 * half, 512 * half + 512)
                    qp = ps_a.tile([MB, 512], f32, tag="psa")
                    for u in range(CT):
                        nc.tensor.matmul(
                            qp[:],
                            qTq[:, u, :],
                            wqT[:, u, nsl],
                            start=(u == 0),
                            stop=(u == CT - 1),
                        )
                    nc.scalar.copy(out=q_sb[:, nsl], in_=qp[:])

                # qT (padded d-major) fp32 via PE transpose
                qT = qtmp.tile([128, H, MB], f32)
                for hd in range(H):
                    trq = ps_a.tile([128, MB], f32, tag="psa")
                    nc.tensor.matmul(
                        trq[:],
                        q_sb[:, 128 * hd : 128 * hd + 128],
                        ident128_32[:],
                        is_transpose=True,
                        start=True,
                        stop=True,
                    )
                    nc.scalar.copy(out=qT[:, hd, :], in_=trq[:])

                # Wk natural fp32 (d-padded rows)
                wk_nat = qtmp.tile([128, H, C], f32)
                nc.vector.memset(wk_nat[HD:128, :, :], 0.0)
                for hd in range(H):
                    nc.sync.dma_start(
                        out=wk_nat[0 : HD // 2, hd, :],
                        in_=wk_d[HD * hd : HD * hd + HD // 2, :],
                    )
                    nc.scalar.dma_start(
                        out=wk_nat[HD // 2 : HD, hd, :],
                        in_=wk_d[HD * hd + HD // 2 : HD * hd + HD, :],
                    )

                # Y: fp32 matmuls packed over both batches (N=128), head
                # quads per PSUM group; psum cols = (head-in-quad, b, n)
                ylr_tiles = [
                    ylrp.tile([128, CT, 512], f16, tag="ylr", name=f"ylr{bb}")
                    for bb in range(b_sh)
                ]
                for u in range(CT):
                    csl = slice(128 * u, 128 * u + 128)
                    for hq in range(2):
                        yp = ps_a.tile([128, 512], f32, tag="psa")
                        for hh in range(4):
                            hd = 4 * hq + hh
                            nc.tensor.matmul(
                                yp[:, 128 * hh : 128 * hh + 128],
                                wk_nat[:, hd, csl],
                                qT[:, hd, :],
                                start=(hh == 0),
                                stop=(hh == 3),
                            )
                        yp4 = yp[:].rearrange("p (hh b n) -> p hh b n", hh=4, b=b_sh)
                        for bb in range(b_sh):
                            dsl = slice(NG * 4 * hq, NG * 4 * hq + NG * 4)
                            nc.scalar.copy(out=Yh[:, bb, u, dsl], in_=yp4[:, :, bb, :])
                            nc.vector.tensor_tensor(
                                out=ylr_tiles[bb][:, u, dsl].rearrange(
                                    "p (hh n) -> p hh n", hh=4
                                ),
                                in0=yp4[:, :, bb, :],
                                in1=Yh[:, bb, u, dsl].rearrange(
                                    "p (hh n) -> p hh n", hh=4
                                ),
                                op=mybir.AluOpType.subtract,
                            )
                for bb in range(b_sh):
                    nc.scalar.mul(
                        out=y8a[:, bb, :, :], in_=Yh[:, bb, :, :], mul=1.0 / 16.0
                    )
                    nc.scalar.mul(out=y8b[:, bb, :, :], in_=ylr_tiles[bb][:], mul=16.0)

                # Wv / Wo transposed fp16 (single term)
                for w_dram, dst in ((wv_d, wvT_h), (wo_d, woT_h)):
                    for t in range(CT):
                        wnat2 = wtmp.tile([128, C], f32, tag="wnat")
                        nc.sync.dma_start(
                            out=wnat2[0:64, :], in_=w_dram[128 * t : 128 * t + 64, :]
                        )
                        nc.scalar.dma_start(
                            out=wnat2[64:128, :],
                            in_=w_dram[128 * t + 64 : 128 * t + 128, :],
                        )
                        whi = wtmp.tile([128, C], f16, tag="whi")
                        nc.vector.tensor_copy(whi[:], wnat2[:])
                        trp16 = ps_tr.tile([128, CT, 128], f16, tag="pstr")
                        for u in range(CT):
                            nc.tensor.matmul(
                                trp16[:, u, :],
                                whi[:, 128 * u : 128 * u + 128],
                                ident128_16[:],
                                is_transpose=True,
                                start=(u == 0),
                                stop=(u == CT - 1),
                            )
                        nc.scalar.copy(out=dst[:, :, 128 * t : 128 * t + 128], in_=trp16[:])

            # =============== phase 2: per-batch S loop ===============
            _es = ExitStack()
            k16p = _es.enter_context(tc.tile_pool(name="k16p", bufs=1))
            ohp = _es.enter_context(tc.tile_pool(name="ohp", bufs=33))
            outp = _es.enter_context(tc.tile_pool(name="outp", bufs=1))
            ga_tiles = []
            recs_tiles = []
            gaT = outp.tile([128, CT, H, MB], f16, tag="gaT")
            for b in range(b_sh):
                for c0 in range(min(3, n_chunks)):
                    fetch_chunk(b, c0)
                k16 = k16p.tile([128, NSUB, 769], f16, tag="k16")
                nc.vector.memset(k16[:, :, 0], 1.0)
                gsr = [
                    ps_g4.tile([128, 509], f32, tag="g4", name=f"gsr{b}_{j}")
                    for j in range(4)
                ]

                def prep_subtile(ig):
                    chunk, i = divmod(ig, n_sub)
                    if i == 0:
                        fetch_chunk(b, chunk + 3)
                    return prep_subtile_core(b, knat_tiles[(b, chunk)], i, ig, k16)

                preps = {}
                warm_cast_pending = set()
                if b == 0:
                    preps.update(warm_preps)
                    warm_cast_pending = set(warm_preps)
                    # fetches the warm preps skipped (one per chunk started)
                    for wig in sorted(warm_preps):
                        chunk, i = divmod(wig, n_sub)
                        if i == 0:
                            fetch_chunk(b, chunk + 3)
                for ig in range(min(2, NSUB)):
                    if ig not in preps:
                        preps[ig] = prep_subtile(ig)
                oh_tiles = []
                for ig in range(NSUB):
                    if ig + 2 < NSUB and ig + 2 not in preps:
                        preps[ig + 2] = prep_subtile(ig + 2)
                    kTh, klr8, kr8 = preps.pop(ig)
                    # logits: 6 fp16 main + 6 fp8 DoubleRow correction matmuls
                    lg = ps_a.tile([128, 512], f32, tag="psa")
                    for u in range(CT):
                        nc.tensor.matmul(
                            lg[:],
                            kTh[:, u, :],
                            Yh[:, b, u, :],
                            start=(u == 0),
                            stop=False,
                        )
                    for t in range(CT // 2):
                        nc.tensor.matmul(
                            lg[:],
                            klr8[:, 2 * t : 2 * t + 2, :],
                            y8a[:, b, 2 * t : 2 * t + 2, :],
                            start=False,
                            stop=False,
                            perf_mode=DRMODE,
                        )
                    for t in range(CT // 2):
                        nc.tensor.matmul(
                            lg[:],
                            kr8[:, 2 * t : 2 * t + 2, :],
                            y8b[:, b, 2 * t : 2 * t + 2, :],
                            start=False,
                            stop=(t == CT // 2 - 1),
                            perf_mode=DRMODE,
                        )
                    # argmax -> one-hot via (x >= rowmax)
                    mx = mxp.tile([128, H], f32, tag="mx")
                    lg3 = lg[:].rearrange("p (h n) -> p h n", h=H)
                    nc.vector.tensor_reduce(
                        out=mx[:],
                        in_=lg3,
                        axis=mybir.AxisListType.X,
                        op=mybir.AluOpType.max,
                    )
                    oh = ohp.tile([128, H * NG], f16, tag="oh")
                    nc.vector.tensor_tensor(
                        out=oh[:].rearrange("p (h n) -> p h n", h=H),
                        in0=lg3,
                        in1=mx[:].unsqueeze(2).to_broadcast((128, H, NG)),
                        op=mybir.AluOpType.is_ge,
                    )
                    oh_tiles.append(oh)
                    if ig in warm_cast_pending:
                        warm_cast_pending.discard(ig)
                        chunk, i = divmod(ig, n_sub)
                        nc.vector.tensor_copy(
                            k16[:, ig, 1:769], knat_tiles[(b, chunk)][:, i, :]
                        )
                    # group-sum pass 1 is emitted ONE subtile behind the
                    # logits so the argmax chain of subtile ig hides under
                    # logits(ig+1) instead of stalling the in-order PE queue
                    for jg in ([ig - 1] if ig > 0 else []) + (
                        [ig] if ig == NSUB - 1 else []
                    ):
                        for j in range(4):
                            nc.tensor.matmul(
                                gsr[j][:],
                                oh_tiles[jg][:, 128 * j : 128 * j + 128],
                                k16[:, jg, 0:509],
                                start=(jg == 0),
                                stop=(jg == NSUB - 1),
                            )

                # recip of counts; divide pass-1 part into ga
                cnts = outp.tile([128, 4], f32, tag="cnts", name=f"cnts{b}")
                recs = outp.tile([128, 4], f32, tag="recs", name=f"recs{b}")
                ga = outp.tile([128, 4, C], f16, tag="ga", name=f"ga{b}")
                for j in range(4):
                    nc.vector.tensor_scalar(
                        out=cnts[:, j : j + 1], in0=gsr[j][:, 0:1],
                        scalar1=1.0, scalar2=None, op0=mybir.AluOpType.add,
                    )
                    nc.vector.reciprocal(recs[:, j : j + 1], cnts[:, j : j + 1])
                    nc.vector.tensor_scalar(
                        out=ga[:, j, 0:508], in0=gsr[j][:, 1:509],
                        scalar1=recs[:, j : j + 1], scalar2=None,
                        op0=mybir.AluOpType.mult,
                    )

                # group-sum pass 2 from retained fp16 key: c[508:768]
                gsr2 = [
                    ps_g4.tile([128, 260], f32, tag="g4", name=f"gsr2_{b}_{j}")
                    for j in range(4)
                ]
                for ig in range(NSUB):
                    for j in range(4):
                        nc.tensor.matmul(
                            gsr2[j][:, 0:260],
                            oh_tiles[ig][:, 128 * j : 128 * j + 128],
                            k16[:, ig, 509:769],
                            start=(ig == 0),
                            stop=(ig == NSUB - 1),
                        )
                for j in range(4):
                    nc.vector.tensor_scalar(
                        out=ga[:, j, 508:768], in0=gsr2[j][:, 0:260],
                        scalar1=recs[:, j : j + 1], scalar2=None,
                        op0=mybir.AluOpType.mult,
                    )
                ga_tiles.append(ga)
                recs_tiles.append(recs)

            # =============== phase 3: output, both batches packed ===============
            # transpose divided group means: gaT[c, u, head, (b n)]
            for b in range(b_sh):
                for j in range(4):
                    trp16 = ps_tr.tile([128, CT, 128], f16, tag="pstr")
                    for u in range(CT):
                        nc.tensor.matmul(
                            trp16[:, u, :],
                            ga_tiles[b][:, j, 128 * u : 128 * u + 128],
                            ident128_16[:],
                            is_transpose=True,
                            start=(u == 0),
                            stop=(u == CT - 1),
                        )
                    nc.scalar.copy(
                        out=gaT[:, :, 2 * j, NG * b : NG * b + NG],
                        in_=trp16[:, :, 0:NG],
                    )
                    nc.scalar.copy(
                        out=gaT[:, :, 2 * j + 1, NG * b : NG * b + NG],
                        in_=trp16[:, :, NG:128],
                    )

            # v-projection of group means, M=128 packed
            attn16 = outp.tile([MB, C], f16, tag="attn16")
            for hd in range(H):
                pa = ps_a.tile([MB, HD], f32, tag="psa")
                for u in range(CT):
                    nc.tensor.matmul(
                        pa[:],
                        gaT[:, u, hd, :],
                        wvT_h[:, u, HD * hd : HD * hd + HD],
                        start=(u == 0),
                        stop=(u == CT - 1),
                    )
                nc.scalar.copy(out=attn16[:, HD * hd : HD * hd + HD], in_=pa[:])

            attnT = outp.tile([128, CT, MB], f16, tag="attnT")
            trp16 = ps_tr.tile([128, CT, 128], f16, tag="pstr")
            for u in range(CT):
                nc.tensor.matmul(
                    trp16[:, u, :],
                    attn16[:, 128 * u : 128 * u + 128],
                    ident128_16[:],
                    is_transpose=True,
                    start=(u == 0),
                    stop=(u == CT - 1),
                )
            nc.scalar.copy(out=attnT[:], in_=trp16[:])

            # Wo projection in column halves; each half's output rows DMA out
            # immediately, split per batch across both HWDGE rings, so the
            # final-store tail is parallel instead of two serial
            # row-descriptor-bound transfers on one ring.
            out_sb = outp.tile([MB, C], f32, tag="outsb")
            for half in range(2):
                nsl = slice(384 * half, 384 * half + 384)
                op = ps_a.tile([MB, 384], f32, tag="psa")
                for u in range(CT):
                    nc.tensor.matmul(
                        op[:],
                        attnT[:, u, :],
                        woT_h[:, u, nsl],
                        start=(u == 0),
                        stop=(u == CT - 1),
                    )
                nc.vector.tensor_tensor(
                    out=out_sb[:, nsl],
                    in0=op[:],
                    in1=bo_bc[:, nsl],
                    op=mybir.AluOpType.add,
                )
                for b in range(b_sh):
                    eng = nc.sync if b % 2 == 0 else nc.scalar
                    eng.dma_start(
                        out=out_d[b, :, nsl],
                        in_=out_sb[NG * b : NG * b + NG, nsl],
                    )
            _es.close()
            kTp.release()

    nc.finalize()
    return nc


_NC_CACHE = {}


def _get_nc(b_sh, S):
    key = (b_sh, S)
    if key not in _NC_CACHE:
        _NC_CACHE[key] = build_nc(b_sh, S)
    return _NC_CACHE[key]


def kernel(query, key_in, Wq, Wk, Wv, Wo, bo):
    from concourse.bass_utils import run_bass_kernel_spmd

    query = np.ascontiguousarray(np.asarray(query, dtype=np.float32))
    key_in = np.ascontiguousarray(np.asarray(key_in, dtype=np.float32))
    Wq = np.ascontiguousarray(np.asarray(Wq, dtype=np.float32))
    Wk = np.ascontiguousarray(np.asarray(Wk, dtype=np.float32))
    Wv = np.ascontiguousarray(np.asarray(Wv, dtype=np.float32))
    Wo = np.ascontiguousarray(np.asarray(Wo, dtype=np.float32))
    bo = np.ascontiguousarray(np.asarray(bo, dtype=np.float32))

    B, _, _ = query.shape
    S = key_in.shape[1]
    n_cores = 8
    b_sh = B // n_cores
    nc = _get_nc(b_sh, S)

    in_maps = []
    for i in range(n_cores):
        bs = slice(i * b_sh, (i + 1) * b_sh)
        in_maps.append(
            {
                "query": np.ascontiguousarray(query[bs]),
                "key_in": np.ascontiguousarray(key_in[bs]),
                "Wq": Wq,
                "Wk": Wk,
                "Wv": Wv,
                "Wo": Wo,
                "bo": bo,
            }
        )
    res = run_bass_kernel_spmd(nc, in_maps, core_ids=list(range(n_cores)))
    out = np.concatenate([res.results[i]["out"] for i in range(n_cores)], axis=0)
    return out.astype(np.float32)


if __name__ == "__main__":
    nc = build_nc(2, 1024)
    print("built ok")


# revision 41
# speedup vs baseline: 1.1751x; 1.1751x over previous
"""AssignAttention (hard-routing slot attention) Trainium2 kernel, 8-core data-parallel.

Problem: B=16, N=64 groups, S=4096 tokens, C=768, H=8 heads, HD=96.
  q = query @ Wq.T; k = key @ Wk.T (per-head split)
  logits = q @ k.T; hard-argmax over the 64 groups per token -> one-hot
  (softmax and the *SCALE factor are argmax-invariant, so both are skipped);
  attn = onehot / (count + 1); out = (attn @ v per head) @ Wo.T + bo

Sharding: data-parallel over batch B: 16 batches / 8 cores = 2 per core.
No collectives; the host concatenates per-core outputs.

Algorithm per core (validated vs the fp32 reference):
  - Logits REASSOCIATED: Y[c, (h,n)] = sum_d Wk[d(h), c] qp[n, d] is tiny
    (per batch 768x512), and logits[s, (h,n)] = sum_c keyT[c, s] Y[c, (h,n)]
    -- the dominant k-projection matmul disappears entirely.
  - Logits precision: fp16 main term (keeps FWL weight loads, which fp32r
    weights would disable) plus TWO fp8e5m2 DoubleRow correction matmuls
    capturing the fp16 rounding residuals of each operand:
      K@Y ~= Kh@Yh + (16*Kl)@(Yh/16) + (Kh/16)@(16*Yl)
    where Kh = fp16(K), Kl = fp16(K - Kh) etc. The +-2^4 pre-scaling keeps
    the e5m2 operands in normal range (subnormal flush would cost argmax
    flips). 6 fp16 MMs + 6 DR MMs (K=256 packed pairs of c-tiles) per
    128-token subtile vs 18 fp16 MMs for the hi/lo x3 split,
    logit rel err ~4e-5 (~27 argmax flips over the whole problem, ~5e-3
    final rel_l2 vs the 2e-2 gate; the exact fp32 q-projection buys extra
    margin over the fp16x3-q sim).
  - Per-subtile operand prep (PE fp32 transpose -> ACT fp16 copy ->
    DVE residual subtract -> ACT scaled fp8 casts) is software-pipelined
    two subtiles ahead of the logits matmuls (three warm-started before
    the weight phase) so the PE never waits on the ACT/DVE chain.
  - argmax via row-max + (x >= max) compare; counts via a ones-column in
    the group-sum rhs; renorm = per-partition reciprocal. Group-sum pass 1
    is emitted one subtile behind the logits so the argmax chain hides
    under the next subtile's matmuls instead of stalling the in-order PE
    queue (PSUM accumulation is add-commutative).
  - v-path reassociated: group-sums taken on raw fp16 keys
    (gs_raw[n,c] = onehot^T @ key16), divided by (count+1), and only the
    64 group vectors are projected through WvT. The fp16 natural-layout
    key (with a leading ones column) is RETAINED in SBUF for the whole
    batch, so group-sum pass 2 (c half two) needs no HBM re-stream and no
    re-cast.
  - Both batches are PACKED into M=128 wherever the row dim would be 64:
    q path (q/qT/q-proj), Y (head-quad PSUM groups over both batches), the
    v-projection and Wo projection, halving the small-matmul count. The
    bias is a partition-broadcast DMA tile folded into the PSUM->SBUF move
    on the DVE (no K=1 matmuls); output stores are split per batch x
    column-half across both HWDGE rings and issued as each half lands.
  - Weight DMAs split across the sync/scalar rings; first three key
    chunks prefetched on the gpsimd (SWDGE) queue during weight prep.
  - One accumulation group per 2KB PSUM bank; all matmul row offsets 0.
"""

import sys

if "/opt/trn_rl_repo" not in sys.path:
    sys.path.insert(0, "/opt/trn_rl_repo")

import numpy as np

import concourse.bass as bass
import concourse.mybir as mybir
from concourse import bacc
import concourse.tile as tile
from concourse.masks import make_identity

f32 = mybir.dt.float32
f32r = mybir.dt.float32r
f16 = mybir.dt.float16
f8 = mybir.dt.float8e5
DRMODE = mybir.MatmulPerfMode.DoubleRow

C = 768
H = 8
HD = 96
NG = 64  # groups
CT = C // 128  # 6 c-tiles
CP = 128 * H  # d-padded width (head h at 128h..128h+96)
S_CHUNK = 256


def build_nc(b_sh=2, S=4096):
    nc = bacc.Bacc()

    query_d = nc.declare_dram_parameter("query", [b_sh, NG, C], f32, isOutput=False)
    key_d = nc.declare_dram_parameter("key_in", [b_sh, S, C], f32, isOutput=False)
    wq_d = nc.declare_dram_parameter("Wq", [C, C], f32, isOutput=False)
    wk_d = nc.declare_dram_parameter("Wk", [C, C], f32, isOutput=False)
    wv_d = nc.declare_dram_parameter("Wv", [C, C], f32, isOutput=False)
    wo_d = nc.declare_dram_parameter("Wo", [C, C], f32, isOutput=False)
    bo_d = nc.declare_dram_parameter("bo", [C], f32, isOutput=False)
    out_d = nc.declare_dram_parameter("out", [b_sh, NG, C], f32, isOutput=True)

    n_chunks = S // S_CHUNK
    n_sub = S_CHUNK // 128
    NSUB = S // 128
    MB = b_sh * NG  # 128 packed rows (2 batches x 64 groups)
    ENGS = None  # set inside

    from contextlib import ExitStack

    with tile.TileContext(nc) as tc:
        with (
            tc.tile_pool(name="wconst", bufs=1) as wconst,
            tc.tile_pool(name="kin", bufs=4) as kin,
            tc.tile_pool(name="mxp", bufs=3) as mxp,
            tc.tile_pool(name="ps_a", bufs=2, space="PSUM") as ps_a,
            tc.tile_pool(name="ps_tr", bufs=2, space="PSUM") as ps_tr,
            tc.tile_pool(name="ps_g4", bufs=4, space="PSUM") as ps_g4,
        ):
            ENGS = (nc.sync, nc.scalar, nc.gpsimd)
            # ---- constants ----
            ident128_32 = wconst.tile([128, 128], f32)
            make_identity(nc, ident128_32[:])
            ident128_16 = wconst.tile([128, 128], f16)
            make_identity(nc, ident128_16[:])
            bo_bc = wconst.tile([128, C], f32)
            nc.gpsimd.dma_start(
                out=bo_bc[:], in_=bo_d[:].unsqueeze(0).to_broadcast((128, C))
            )

            # persistent weight / Y tiles
            wvT_h = wconst.tile([128, CT, C], f16)
            woT_h = wconst.tile([128, CT, C], f16)
            Yh = wconst.tile([128, b_sh, CT, 512], f16)
            y8a = wconst.tile([128, b_sh, CT, 512], f8)
            y8b = wconst.tile([128, b_sh, CT, 512], f8)

            # prefetch first key chunks of batch 0 on the SWDGE queue
            knat_tiles = {}
            def fetch_chunk(b, c):
                if (b, c) in knat_tiles or c >= n_chunks:
                    return
                t = kin.tile([128, n_sub, C], f32, tag="knat", name=f"knat_{b}_{c}")
                nc.gpsimd.dma_start(
                    out=t[:],
                    in_=key_d[b, c * S_CHUNK : (c + 1) * S_CHUNK, :].rearrange(
                        "(i p) c -> p i c", p=128
                    ),
                )
                knat_tiles[(b, c)] = t

            for c0 in range(min(4, n_chunks)):
                fetch_chunk(0, c0)

            # per-subtile logits-operand prep; kTp is open before phase 1 so
            # the first subtiles' transposes/casts can run during phase-1
            # weight-DMA stalls (k16 cast deferred for those warm subtiles)
            kTp = tc.alloc_tile_pool(name="kTp", bufs=3)

            def prep_subtile_core(b, knat, i, ig, k16):
                kTh = kTp.tile([128, CT, 128], f16, tag="kTh")
                klr16 = kTp.tile([128, CT, 128], f16, tag="klr16")
                for g in range(2):
                    trp = ps_tr.tile([128, 3, 128], f32, tag="pstr")
                    for j in range(3):
                        u = 3 * g + j
                        nc.tensor.matmul(
                            trp[:, j, :],
                            knat[:, i, 128 * u : 128 * u + 128],
                            ident128_32[:],
                            is_transpose=True,
                            start=(j == 0),
                            stop=(j == 2),
                        )
                    nc.scalar.copy(out=kTh[:, 3 * g : 3 * g + 3, :], in_=trp[:])
                    nc.vector.tensor_tensor(
                        out=klr16[:, 3 * g : 3 * g + 3, :],
                        in0=trp[:],
                        in1=kTh[:, 3 * g : 3 * g + 3, :],
                        op=mybir.AluOpType.subtract,
                    )
                klr8 = kTp.tile([128, CT, 128], f8, tag="klr8")
                nc.scalar.mul(out=klr8[:], in_=klr16[:], mul=16.0)
                kr8 = kTp.tile([128, CT, 128], f8, tag="kr8")
                nc.scalar.mul(out=kr8[:], in_=kTh[:], mul=1.0 / 16.0)
                if k16 is not None:
                    nc.vector.tensor_copy(k16[:, ig, 1:769], knat[:, i, :])
                return kTh, klr8, kr8

            # warm preps: batch-0 subtiles 0..2 (k16 cast deferred into the
            # loop so the casts don't queue on DVE ahead of the first argmax)
            warm_preps = {}
            for wig in range(min(3, NSUB)):
                chunk, i = divmod(wig, n_sub)
                warm_preps[wig] = prep_subtile_core(0, knat_tiles[(0, chunk)], i, wig, None)

            # =============== phase 1: weights + q path + Y ===============
            with (
                tc.tile_pool(name="qtmp", bufs=1) as qtmp,
                tc.tile_pool(name="wtmp", bufs=5) as wtmp,
                tc.tile_pool(name="ylrp", bufs=2) as ylrp,
            ):
                # Wq transposed to c-major fp32, d-padded
                wqT = qtmp.tile([128, CT, CP], f32)
                for hd in range(H):
                    wnat = wtmp.tile([128, C], f32, tag="wnat")
                    nc.vector.memset(wnat[HD:128, :], 0.0)
                    nc.sync.dma_start(
                        out=wnat[0 : HD // 2, :], in_=wq_d[HD * hd : HD * hd + HD // 2, :]
                    )
                    nc.scalar.dma_start(
                        out=wnat[HD // 2 : HD, :],
                        in_=wq_d[HD * hd + HD // 2 : HD * hd + HD, :],
                    )
                    for g in range(2):
                        trp = ps_tr.tile([128, 3, 128], f32, tag="pstr")
                        for j in range(3):
                            u = 3 * g + j
                            nc.tensor.matmul(
                                trp[:, j, :],
                                wnat[:, 128 * u : 128 * u + 128],
                                ident128_32[:],
                                is_transpose=True,
                                start=(j == 0),
                                stop=(j == 2),
                            )
                        nc.scalar.copy(
                            out=wqT[:, 3 * g : 3 * g + 3, 128 * hd : 128 * hd + 128],
                            in_=trp[:],
                        )

                # query (both batches packed), transposed to c-major fp32
                q_nat = qtmp.tile([MB, C], f32)
                qflat = query_d[:].rearrange("b n c -> (b n) c")
                nc.sync.dma_start(out=q_nat[0:64, :], in_=qflat[0:64, :])
                nc.scalar.dma_start(out=q_nat[64:128, :], in_=qflat[64:128, :])
                qTq = qtmp.tile([128, CT, MB], f32)
                for g in range(2):
                    trp = ps_tr.tile([128, 3, 128], f32, tag="pstr")
                    for j in range(3):
                        u = 3 * g + j
                        nc.tensor.matmul(
                            trp[:, j, :],
                            q_nat[:, 128 * u : 128 * u + 128],
                            ident128_32[:],
                            is_transpose=True,
                            start=(j == 0),
                            stop=(j == 2),
                        )
                    nc.scalar.copy(out=qTq[:, 3 * g : 3 * g + 3, :], in_=trp[:])

                # q projection fp32 direct (exact), M=128 packed
                q_sb = qtmp.tile([MB, CP], f32)
                for half in range(2):
                    nsl = slice(512 * half, 512 * half + 512)
                    qp = ps_a.tile([MB, 512], f32, tag="psa")
                    for u in range(CT):
                        nc.tensor.matmul(
                            qp[:],
                            qTq[:, u, :],
                            wqT[:, u, nsl],
                            start=(u == 0),
                            stop=(u == CT - 1),
                        )
                    nc.scalar.copy(out=q_sb[:, nsl], in_=qp[:])

                # qT (padded d-major) fp32 via PE transpose
                qT = qtmp.tile([128, H, MB], f32)
                for hd in range(H):
                    trq = ps_a.tile([128, MB], f32, tag="psa")
                    nc.tensor.matmul(
                        trq[:],
                        q_sb[:, 128 * hd : 128 * hd + 128],
                        ident128_32[:],
                        is_transpose=True,
                        start=True,
                        stop=True,
                    )
                    nc.scalar.copy(out=qT[:, hd, :], in_=trq[:])

                # Wk natural fp32 (d-padded rows)
                wk_nat = qtmp.tile([128, H, C], f32)
                nc.vector.memset(wk_nat[HD:128, :, :], 0.0)
                for hd in range(H):
                    nc.sync.dma_start(
                        out=wk_nat[0 : HD // 2, hd, :],
                        in_=wk_d[HD * hd : HD * hd + HD // 2, :],
                    )
                    nc.scalar.dma_start(
                        out=wk_nat[HD // 2 : HD, hd, :],
                        in_=wk_d[HD * hd + HD // 2 : HD * hd + HD, :],
                    )

                # Y: fp32 matmuls packed over both batches (N=128), head
                # quads per PSUM group; psum cols = (head-in-quad, b, n)
                ylr_tiles = [
                    ylrp.tile([128, CT, 512], f16, tag="ylr", name=f"ylr{bb}")
                    for bb in range(b_sh)
                ]
                for u in range(CT):
                    csl = slice(128 * u, 128 * u + 128)
                    for hq in range(2):
                        yp = ps_a.tile([128, 512], f32, tag="psa")
                        for hh in range(4):
                            hd = 4 * hq + hh
                            nc.tensor.matmul(
                                yp[:, 128 * hh : 128 * hh + 128],
                                wk_nat[:, hd, csl],
                                qT[:, hd, :],
                                start=(hh == 0),
                                stop=(hh == 3),
                            )
                        yp4 = yp[:].rearrange("p (hh b n) -> p hh b n", hh=4, b=b_sh)
                        for bb in range(b_sh):
                            dsl = slice(NG * 4 * hq, NG * 4 * hq + NG * 4)
                            nc.scalar.copy(out=Yh[:, bb, u, dsl], in_=yp4[:, :, bb, :])
                            nc.vector.tensor_tensor(
                                out=ylr_tiles[bb][:, u, dsl].rearrange(
                                    "p (hh n) -> p hh n", hh=4
                                ),
                                in0=yp4[:, :, bb, :],
                                in1=Yh[:, bb, u, dsl].rearrange(
                                    "p (hh n) -> p hh n", hh=4
                                ),
                                op=mybir.AluOpType.subtract,
                            )
                for bb in range(b_sh):
                    nc.scalar.mul(
                        out=y8a[:, bb, :, :], in_=Yh[:, bb, :, :], mul=1.0 / 16.0
                    )
                    nc.scalar.mul(out=y8b[:, bb, :, :], in_=ylr_tiles[bb][:], mul=16.0)

                # Wv / Wo transposed fp16 (single term)
                for w_dram, dst in ((wv_d, wvT_h), (wo_d, woT_h)):
                    for t in range(CT):
                        wnat2 = wtmp.tile([128, C], f32, tag="wnat")
                        nc.sync.dma_start(
                            out=wnat2[0:64, :], in_=w_dram[128 * t : 128 * t + 64, :]
                        )
                        nc.scalar.dma_start(
                            out=wnat2[64:128, :],
                            in_=w_dram[128 * t + 64 : 128 * t + 128, :],
                        )
                        whi = wtmp.tile([128, C], f16, tag="whi")
                        nc.vector.tensor_copy(whi[:], wnat2[:])
                        trp16 = ps_tr.tile([128, CT, 128], f16, tag="pstr")
                        for u in range(CT):
                            nc.tensor.matmul(
                                trp16[:, u, :],
                                whi[:, 128 * u : 128 * u + 128],
                                ident128_16[:],
                                is_transpose=True,
                                start=(u == 0),
                                stop=(u == CT - 1),
                            )
                        nc.scalar.copy(out=dst[:, :, 128 * t : 128 * t + 128], in_=trp16[:])

            # =============== phase 2: per-batch S loop ===============
            _es = ExitStack()
            k16p = _es.enter_context(tc.tile_pool(name="k16p", bufs=1))
            ohp = _es.enter_context(tc.tile_pool(name="ohp", bufs=33))
            outp = _es.enter_context(tc.tile_pool(name="outp", bufs=1))
            ga_tiles = []
            recs_tiles = []
            for b in range(b_sh):
                for c0 in range(min(3, n_chunks)):
                    fetch_chunk(b, c0)
                k16 = k16p.tile([128, NSUB, 769], f16, tag="k16")
                nc.vector.memset(k16[:, :, 0], 1.0)
                gsr = [
                    ps_g4.tile([128, 509], f32, tag="g4", name=f"gsr{b}_{j}")
                    for j in range(4)
                ]

                def prep_subtile(ig):
                    chunk, i = divmod(ig, n_sub)
                    if i == 0:
                        fetch_chunk(b, chunk + 3)
                    return prep_subtile_core(b, knat_tiles[(b, chunk)], i, ig, k16)

                preps = {}
                warm_cast_pending = set()
                if b == 0:
                    preps.update(warm_preps)
                    warm_cast_pending = set(warm_preps)
                    # fetches the warm preps skipped (one per chunk started)
                    for wig in sorted(warm_preps):
                        chunk, i = divmod(wig, n_sub)
                        if i == 0:
                            fetch_chunk(b, chunk + 3)
                for ig in range(min(2, NSUB)):
                    if ig not in preps:
                        preps[ig] = prep_subtile(ig)
                oh_tiles = []
                for ig in range(NSUB):
                    if ig + 2 < NSUB and ig + 2 not in preps:
                        preps[ig + 2] = prep_subtile(ig + 2)
                    kTh, klr8, kr8 = preps.pop(ig)
                    # logits: 6 fp16 main + 6 fp8 DoubleRow correction matmuls
                    lg = ps_a.tile([128, 512], f32, tag="psa")
                    for u in range(CT):
                        nc.tensor.matmul(
                            lg[:],
                            kTh[:, u, :],
                            Yh[:, b, u, :],
                            start=(u == 0),
                            stop=False,
                        )
                    for t in range(CT // 2):
                        nc.tensor.matmul(
                            lg[:],
                            klr8[:, 2 * t : 2 * t + 2, :],
                            y8a[:, b, 2 * t : 2 * t + 2, :],
                            start=False,
                            stop=False,
                            perf_mode=DRMODE,
                        )
                    for t in range(CT // 2):
                        nc.tensor.matmul(
                            lg[:],
                            kr8[:, 2 * t : 2 * t + 2, :],
                            y8b[:, b, 2 * t : 2 * t + 2, :],
                            start=False,
                            stop=(t == CT // 2 - 1),
                            perf_mode=DRMODE,
                        )
                    # argmax -> one-hot via (x >= rowmax)
                    mx = mxp.tile([128, H], f32, tag="mx")
                    lg3 = lg[:].rearrange("p (h n) -> p h n", h=H)
                    nc.vector.tensor_reduce(
                        out=mx[:],
                        in_=lg3,
                        axis=mybir.AxisListType.X,
                        op=mybir.AluOpType.max,
                    )
                    oh = ohp.tile([128, H * NG], f16, tag="oh")
                    nc.vector.tensor_tensor(
                        out=oh[:].rearrange("p (h n) -> p h n", h=H),
                        in0=lg3,
                        in1=mx[:].unsqueeze(2).to_broadcast((128, H, NG)),
                        op=mybir.AluOpType.is_ge,
                    )
                    oh_tiles.append(oh)
                    if ig in warm_cast_pending:
                        warm_cast_pending.discard(ig)
                        chunk, i = divmod(ig, n_sub)
                        nc.vector.tensor_copy(
                            k16[:, ig, 1:769], knat_tiles[(b, chunk)][:, i, :]
                        )
                    # group-sum pass 1 is emitted ONE subtile behind the
                    # logits so the argmax chain of subtile ig hides under
                    # logits(ig+1) instead of stalling the in-order PE queue
                    for jg in ([ig - 1] if ig > 0 else []) + (
                        [ig] if ig == NSUB - 1 else []
                    ):
                        for j in range(4):
                            nc.tensor.matmul(
                                gsr[j][:],
                                oh_tiles[jg][:, 128 * j : 128 * j + 128],
                                k16[:, jg, 0:509],
                                start=(jg == 0),
                                stop=(jg == NSUB - 1),
                            )

                # recip of counts; divide pass-1 part into ga
                cnts = outp.tile([128, 4], f32, tag="cnts", name=f"cnts{b}")
                recs = outp.tile([128, 4], f32, tag="recs", name=f"recs{b}")
                ga = outp.tile([128, 4, C], f16, tag="ga", name=f"ga{b}")
                for j in range(4):
                    nc.vector.tensor_scalar(
                        out=cnts[:, j : j + 1], in0=gsr[j][:, 0:1],
                        scalar1=1.0, scalar2=None, op0=mybir.AluOpType.add,
                    )
                    nc.vector.reciprocal(recs[:, j : j + 1], cnts[:, j : j + 1])
                    nc.vector.tensor_scalar(
                        out=ga[:, j, 0:508], in0=gsr[j][:, 1:509],
                        scalar1=recs[:, j : j + 1], scalar2=None,
                        op0=mybir.AluOpType.mult,
                    )

                # group-sum pass 2 from retained fp16 key: c[508:768]
                gsr2 = [
                    ps_g4.tile([128, 260], f32, tag="g4", name=f"gsr2_{b}_{j}")
                    for j in range(4)
                ]
                for ig in range(NSUB):
                    for j in range(4):
                        nc.tensor.matmul(
                            gsr2[j][:, 0:260],
                            oh_tiles[ig][:, 128 * j : 128 * j + 128],
                            k16[:, ig, 509:769],
                            start=(ig == 0),
                            stop=(ig == NSUB - 1),
                        )
                for j in range(4):
                    nc.vector.tensor_scalar(
                        out=ga[:, j, 508:768], in0=gsr2[j][:, 0:260],
                        scalar1=recs[:, j : j + 1], scalar2=None,
                        op0=mybir.AluOpType.mult,
                    )
                ga_tiles.append(ga)
                recs_tiles.append(recs)

            # =============== phase 3: output, both batches packed ===============
            # transpose divided group means: gaT[c, u, head, (b n)]
            gaT = outp.tile([128, CT, H, MB], f16, tag="gaT")
            for b in range(b_sh):
                for j in range(4):
                    trp16 = ps_tr.tile([128, CT, 128], f16, tag="pstr")
                    for u in range(CT):
                        nc.tensor.matmul(
                            trp16[:, u, :],
                            ga_tiles[b][:, j, 128 * u : 128 * u + 128],
                            ident128_16[:],
                            is_transpose=True,
                            start=(u == 0),
                            stop=(u == CT - 1),
                        )
                    nc.scalar.copy(
                        out=gaT[:, :, 2 * j, NG * b : NG * b + NG],
                        in_=trp16[:, :, 0:NG],
                    )
                    nc.scalar.copy(
                        out=gaT[:, :, 2 * j + 1, NG * b : NG * b + NG],
                        in_=trp16[:, :, NG:128],
                    )

            # v-projection of group means, M=128 packed
            attn16 = outp.tile([MB, C], f16, tag="attn16")
            for hd in range(H):
                pa = ps_a.tile([MB, HD], f32, tag="psa")
                for u in range(CT):
                    nc.tensor.matmul(
                        pa[:],
                        gaT[:, u, hd, :],
                        wvT_h[:, u, HD * hd : HD * hd + HD],
                        start=(u == 0),
                        stop=(u == CT - 1),
                    )
                nc.scalar.copy(out=attn16[:, HD * hd : HD * hd + HD], in_=pa[:])

            attnT = outp.tile([128, CT, MB], f16, tag="attnT")
            trp16 = ps_tr.tile([128, CT, 128], f16, tag="pstr")
            for u in range(CT):
                nc.tensor.matmul(
                    trp16[:, u, :],
                    attn16[:, 128 * u : 128 * u + 128],
                    ident128_16[:],
                    is_transpose=True,
                    start=(u == 0),
                    stop=(u == CT - 1),
                )
            nc.scalar.copy(out=attnT[:], in_=trp16[:])

            # Wo projection in column halves; each half's output rows DMA out
            # immediately, split per batch across both HWDGE rings, so the
            # final-store tail is parallel instead of two serial
            # row-descriptor-bound transfers on one ring.
            out_sb = outp.tile([MB, C], f32, tag="outsb")
            for half in range(2):
                nsl = slice(384 * half, 384 * half + 384)
                op = ps_a.tile([MB, 384], f32, tag="psa")
                for u in range(CT):
                    nc.tensor.matmul(
                        op[:],
                        attnT[:, u, :],
                        woT_h[:, u, nsl],
                        start=(u == 0),
                        stop=(u == CT - 1),
                    )
                nc.vector.tensor_tensor(
                    out=out_sb[:, nsl],
                    in0=op[:],
                    in1=bo_bc[:, nsl],
                    op=mybir.AluOpType.add,
                )
                for b in range(b_sh):
                    eng = nc.sync if b % 2 == 0 else nc.scalar
                    eng.dma_start(
                        out=out_d[b, :, nsl],
                        in_=out_sb[NG * b : NG * b + NG, nsl],
                    )
            _es.close()
            kTp.release()

    nc.finalize()
    return nc


_NC_CACHE = {}


def _get_nc(b_sh, S):
    key = (b_sh, S)
    if key not in _NC_CACHE:
        _NC_CACHE[key] = build_nc(b_sh, S)
    return _NC_CACHE[key]


def kernel(query, key_in, Wq, Wk, Wv, Wo, bo):
    from concourse.bass_utils import run_bass_kernel_spmd

    query = np.ascontiguousarray(np.asarray(query, dtype=np.float32))
    key_in = np.ascontiguousarray(np.asarray(key_in, dtype=np.float32))
    Wq = np.ascontiguousarray(np.asarray(Wq, dtype=np.float32))
    Wk = np.ascontiguousarray(np.asarray(Wk, dtype=np.float32))
    Wv = np.ascontiguousarray(np.asarray(Wv, dtype=np.float32))
    Wo = np.ascontiguousarray(np.asarray(Wo, dtype=np.float32))
    bo = np.ascontiguousarray(np.asarray(bo, dtype=np.float32))

    B, _, _ = query.shape
    S = key_in.shape[1]
    n_cores = 8
    b_sh = B // n_cores
    nc = _get_nc(b_sh, S)

    in_maps = []
    for i in range(n_cores):
        bs = slice(i * b_sh, (i + 1) * b_sh)
        in_maps.append(
            {
                "query": np.ascontiguousarray(query[bs]),
                "key_in": np.ascontiguousarray(key_in[bs]),
                "Wq": Wq,
                "Wk": Wk,
                "Wv": Wv,
                "Wo": Wo,
                "bo": bo,
            }
        )
    res = run_bass_kernel_spmd(nc, in_maps, core_ids=list(range(n_cores)))
    out = np.concatenate([res.results[i]["out"] for i in range(n_cores)], axis=0)
    return out.astype(np.float32)


if __name__ == "__main__":
    nc = build_nc(2, 1024)
    print("built ok")
